# revision 15
# baseline (speedup 1.0000x reference)
"""Trainium2 Bass kernel for BailingMoeV2.5 linear attention layer.

Sharding: 8 cores = 2 batches x 4 head-groups. Core c handles batch c//4,
heads 4*(c%4) .. +4 (of 16). Each core computes its head-slice of
qkv/g projections, chunked ALiBi-decayed linear attention, group-RMSNorm,
sigmoid gate, and a partial dense output (its 512 rows of w_dense).
Host sums the 4 partial outputs per batch.

All matmuls in fp16 (1 cyc/row on PE, 10-bit mantissa), fp32 PSUM
accumulation, fp32 recurrent state master.

Layout strategy per core:
  - hiddenT (fp16, [d_in, s]) host-pre-transposed; projections of q,k,v
    token-major (hiddenT tiles stationary), g head-dim-major (W_g stationary).
  - q,k norm+rope in token-major (free-dim reductions), then XBAR
    DMA-transposed per 128-token subtile into head-dim-major [d, si, h, t]
    tiles for the attention matmuls (no PE transposes, no PSUM traffic).
  - attention: per 256-chunk, per head-pair shared PSUM banks; o (intra +
    inter) accumulates [e, i]; group-norm rstd via PE ones-reduction
    batched [P, 4] per chunk; gate in head-dim-major.
  - dense: ogT (fp16) stationary, w_dense moving -> token-major partial
    out; eviction split scalar (psA*rstd0) || vector (psB*rstd1 + tmp),
    one 1KB-per-partition output DMA per group.
  - DMA rings: sync carries w/hT/outputs; scalar carries small consts and
    the XBAR transposes (parallel descriptor programming at startup).
"""

import math
from contextlib import ExitStack

import numpy as np

import concourse.mybir as mybir
import concourse.tile as tile
from concourse import bacc
from concourse.bass_utils import run_bass_kernel_spmd

dt = mybir.dt
F32 = dt.float32
F16 = dt.float16
AL = mybir.AluOpType
AF = mybir.ActivationFunctionType

# static model config
NH, HD, HID = 16, 128, 2048
ROT, HALF = 64, 32
EPS = 1e-6
THETA = 10000.0
LAYER_IDX, N_LAYERS = 1, 32
B, S = 2, 2048

DEBUG_TAPS = False
TAP_SET = ("q", "k", "v", "sg", "og", "st")

# kernel tiling config
NCORES = 8
NHL = 4            # heads per core
CH = 256           # internal chunk length (exact algebraic regrouping)
BLK = 512          # tokens per projection block
NBLK = S // BLK    # 4
SUBS = BLK // 128  # 4 s-subtiles per block
KT = HID // 128    # 16 d_in tiles
DOUT = NHL * HD    # 512 per tensor (q,k,v,g)


def _base_slopes(n):
    start = 2 ** (-(2 ** (-(math.log2(n) - 3))))
    return [start * (start ** i) for i in range(n)]


_SLOPE_ALL = -np.array(_base_slopes(NH), dtype=np.float64) * (
    1.0 - (LAYER_IDX - 1) / (N_LAYERS - 1) + 1e-5
)  # [NH] negative log-decay


def _build_module():
    nc = bacc.Bacc("TRN2", target_bir_lowering=False, debug=False,
                   num_devices=NCORES)

    f16in = lambda name, shape: nc.dram_tensor(
        name, shape, F16, kind="ExternalInput").ap()
    f32in = lambda name, shape: nc.dram_tensor(
        name, shape, F32, kind="ExternalInput").ap()

    d = {
        "hT": f16in("hT", [HID, S]),
        "wqkvg": f16in("wqkvg", [128, KT, 4 * DOUT]),
        "wd": f16in("wd", [128, NHL, HID]),
        "costab": f16in("costab", [128, S // 128, HALF]),
        "sintab": f16in("sintab", [128, S // 128, HALF]),
        "qnw": f16in("qnw", [128, HD]),
        "knw": f16in("knw", [128, HD]),
        "maskt": f16in("maskt", [128, NHL, 2, CH]),
        "onec": f16in("onec", [128, 1]),
        "qdecb": f16in("qdecb", [128, 2, NHL, 128]),
        "kdec": f32in("kdec", [128, NHL, 2]),
        "lamv": f16in("lamv", [128, NHL]),
        "st0": f16in("st0", [128, NHL, HD]),
        "outp": nc.dram_tensor("outp", [S, HID], F16,
                               kind="ExternalOutput").ap(),
    }
    if DEBUG_TAPS:
        for nm, shape, dtp in [("dbg_q", [128, SUBS, DOUT], F16),
                               ("dbg_k", [128, SUBS, DOUT], F16),
                               ("dbg_v", [128, SUBS, DOUT], F16),
                               ("dbg_og", [128, NHL, BLK], F16),
                               ("dbg_st", [128, NHL, HD], F32),
                               ("dbg_sg", [128, NHL, BLK], F16)]:
            d[nm] = nc.dram_tensor(nm, shape, dtp,
                                   kind="ExternalOutput").ap()

    with tile.TileContext(nc) as tc, ExitStack() as ctx, \
            nc.allow_low_precision(reason="fp16 operands, fp32 accumulate"):
        _body(nc, tc, ctx, d)

    nc.compile()
    return nc


def _body(nc, tc, ctx, d):
    P = 128

    pool = lambda name, bufs: ctx.enter_context(
        tc.tile_pool(name=name, bufs=bufs))
    const = pool("const", 1)      # tables, masks, state (~11k)
    wpool = pool("wpool", 1)      # 80k: resident weights (fp16)
    htpool = pool("ht", 3)        # 48k: hT big tiles, 3 blocks resident
    qkp = pool("qkp", 5)          # 10k: q/k token-major
    vp = pool("vp", 8)            # 8k: v token-major, 2 blocks
    sigp = pool("sigp", 2)        # 8k: sigmoid(g) head-dim-major (fp16)
    sqscp = pool("sqsc", 1)       # 1k: sumsq squares scratch (fp16)
    ropep = pool("ropep", 1)      # 1k: rope m1..m4 (fp16)
    ssp = pool("ssp", 6)          # ~0.5k: sumsq/rstd chains (fp32)
    trp = pool("trp", 2)          # 12k: qT/kT/qdT chunk tiles (fp16)
    stp = pool("stp", 2)          # 2k: masked scoresT (fp16)
    smallp = pool("smallp", 3)    # 3k: kdec-scaled v (fp16)
    stcp = pool("stcp", 2)        # 2k: fp16 state snapshot
    stlp = pool("stlp", 2)        # 2k: decayed state (fp16)
    sqp = pool("sqp", 3)          # 3k: o squares (2 alive per chunk)
    gnp = pool("gnp", 2)          # <1k: group-norm rstd chain
    ogp = pool("ogp", 2)          # 8k: ogT fp16 block
    outsp = pool("outs", 3)       # 6k: dense output staging (fp16)

    psum = ctx.enter_context(tc.tile_pool(name="ps", bufs=8, space="PSUM"))
    psn = [0]

    def ps_tile(shape, dtype=F32):
        psn[0] += 1
        return psum.tile(shape, dtype, tag="ps", name=f"ps{psn[0]}")

    def loadc(name, shape, dtype=F16):
        t = const.tile(shape, dtype, tag=name, name=name)
        nc.scalar.dma_start(t[:], d[name])
        return t

    # HAM warmup: real matmuls on a zeroed scratch tile (no DMA dependency)
    # keep the PE busy during the initial weight DMA so the clock-gate opens
    # (1.2 -> 2.4 GHz) before the first real matmul
    scratch = const.tile([P, 256], F16, tag="scratch", name="scratch")
    nc.vector.memset(scratch[:], 0.0)
    wrm1 = ps_tile([P, 256])
    wrm2 = ps_tile([P, 256])
    for i in range(8):
        nc.tensor.matmul(wrm1[:], scratch[:, 0:128], scratch[:],
                         start=(i == 0), stop=(i == 7))
        nc.tensor.matmul(wrm2[:], scratch[:, 0:128], scratch[:],
                         start=(i == 0), stop=(i == 7))

    # big hT tiles: block 0 filled by per-k DMAs interleaved with the weight
    # stream (fine-grained deps for the prologue); blocks 1,2 one-shot
    ht_big = [htpool.tile([P, KT, BLK], F16, tag="htb", name=f"htb{i}")
              for i in range(3)]
    w_tiles = []
    for k in range(KT):
        wt = wpool.tile([P, 4 * DOUT], F16, tag=f"w{k}", name=f"w{k}")
        nc.sync.dma_start(wt[:], d["wqkvg"][:, k, :])
        w_tiles.append(wt)
        nc.sync.dma_start(ht_big[0][:, k, :],
                          d["hT"][k * 128:(k + 1) * 128, 0:BLK])
    # small consts on the scalar ring - parallel descriptor programming,
    # all landed before the first norm/attention consumers
    cos_t = loadc("costab", [P, S // 128, HALF])
    sin_t = loadc("sintab", [P, S // 128, HALF])
    qnw_t = loadc("qnw", [P, HD])
    knw_t = loadc("knw", [P, HD])
    mask_t = loadc("maskt", [P, NHL, 2, CH])
    qdecb_t = loadc("qdecb", [P, 2, NHL, 128])
    kdec_t = loadc("kdec", [P, NHL, 2], F32)
    lamv_t = loadc("lamv", [P, NHL])
    st_c0 = loadc("st0", [P, NHL, HD])
    ones_col = loadc("onec", [P, 1])
    # bulk prefetch of later hT blocks on the sync ring
    def emit_ht_block(nn, buf):
        nc.sync.dma_start(
            buf[:], d["hT"].rearrange("(k p) s -> p k s", p=128)
            [:, :, nn * BLK:(nn + 1) * BLK])
        return [buf[:, k, :] for k in range(KT)]

    ht_blocks = [None] * NBLK
    ht_blocks[0] = [ht_big[0][:, k, :] for k in range(KT)]
    wd_t = wpool.tile([P, NHL, HID], F16, tag="wd", name="wd")

    def make_proj(ht):
        """Allocate a block's q/k/v tiles; return 12 emit-closures (one PSUM
        accumulation group each: 16 matmuls + eviction + q/k sumsq)."""
        q_blk = [qkp.tile([P, DOUT], F16, tag="qb", name="qb") for _ in range(SUBS)]
        k_blk = [qkp.tile([P, DOUT], F16, tag="kb", name="kb") for _ in range(SUBS)]
        v_blk = [vp.tile([P, DOUT], F16, tag="vb", name="vb") for _ in range(SUBS)]
        ss_l = [ssp.tile([P, 8], F32, tag="ss", name="ss") for _ in range(SUBS)]
        groups = []
        for sub in range(SUBS):
            for ti, dest in enumerate((q_blk, k_blk, v_blk)):
                def grp(sub=sub, ti=ti, dest=dest, ss_t=ss_l[sub], ht=ht):
                    ps = ps_tile([P, DOUT])
                    for k in range(KT):
                        nc.tensor.matmul(
                            ps[:], ht[k][:, sub * 128:(sub + 1) * 128],
                            w_tiles[k][:, ti * DOUT:(ti + 1) * DOUT],
                            start=(k == 0), stop=(k == KT - 1))
                    sb = dest[sub]
                    nc.scalar.copy(sb[:], ps[:])
                    if ti < 2:
                        sqs = sqscp.tile([P, DOUT], F16, tag="sqscratch")
                        nc.vector.tensor_mul(sqs[:], sb[:], sb[:])
                        nc.vector.tensor_reduce(
                            ss_t[:, ti * 4:ti * 4 + 4],
                            sqs[:].rearrange("p (h d) -> p h d", h=NHL),
                            mybir.AxisListType.X, AL.add)
                groups.append(grp)
        return groups, (q_blk, k_blk, v_blk, ss_l)

    def emit_norm_rope(n, blk_state, subs_range=None):
        q_blk, k_blk, v_blk, ss_l = blk_state
        for sub in (range(SUBS) if subs_range is None else subs_range):
            gs = n * SUBS + sub
            rtmp = ssp.tile([P, 8], F32, tag="rstdt")
            nc.vector.tensor_scalar(rtmp[:], ss_l[sub][:], 1.0 / HD, EPS,
                                    AL.mult, AL.add)
            nc.vector.reciprocal(rtmp[:], rtmp[:])
            rstd_t = ssp.tile([P, 8], F16, tag="rstd")
            nc.scalar.activation(rstd_t[:], rtmp[:], AF.Sqrt)
            for ti, (blk, nw_t) in enumerate(((q_blk, qnw_t), (k_blk, knw_t))):
                x = blk[sub]
                x3 = x.rearrange("p (h d) -> p h d", h=NHL)
                rsl = rstd_t[:, ti * 4:ti * 4 + 4]
                nc.vector.tensor_mul(
                    x3, x3, rsl.unsqueeze(2).to_broadcast((P, NHL, HD)))
                nc.vector.tensor_mul(
                    x3, x3, nw_t[:].unsqueeze(1).to_broadcast((P, NHL, HD)))
                x1, x2 = x3[:, :, 0:HALF], x3[:, :, HALF:ROT]
                cosb = cos_t[:, gs, :].unsqueeze(1).to_broadcast(
                    (P, NHL, HALF))
                sinb = sin_t[:, gs, :].unsqueeze(1).to_broadcast(
                    (P, NHL, HALF))
                m1 = ropep.tile([P, NHL, HALF], F16, tag="m1")
                m2 = ropep.tile([P, NHL, HALF], F16, tag="m2")
                m3 = ropep.tile([P, NHL, HALF], F16, tag="m3")
                m4 = ropep.tile([P, NHL, HALF], F16, tag="m4")
                nc.vector.tensor_mul(m1[:], x1, cosb)
                nc.vector.tensor_mul(m2[:], x2, sinb)
                nc.vector.tensor_mul(m3[:], x2, cosb)
                nc.vector.tensor_mul(m4[:], x1, sinb)
                nc.vector.tensor_sub(x1, m1[:], m2[:])
                nc.vector.tensor_add(x2, m3[:], m4[:])

    def alloc_chunk_T():
        # [d, si, h, t] head-dim-major chunk tiles, filled by XBAR transposes
        qT = trp.tile([P, 2, NHL, 128], F16, tag="qT", name="qT")
        kT = trp.tile([P, 2, NHL, 128], F16, tag="kT", name="kT")
        return qT, kT

    def emit_xpose(blk_state, cc_tiles, sub):
        # XBAR DMA transpose (scalar ring): q_blk[sub] [t,(h d)] -> [d,h,t]
        q_blk, k_blk, v_blk, ss_l = blk_state
        qT, kT = cc_tiles[sub // 2]
        si = sub % 2
        nc.sync.dma_start_transpose(qT[:, si], q_blk[sub][:])
        nc.scalar.dma_start_transpose(kT[:, si], k_blk[sub][:])

    def dense_group(nn, ogT, rstd_gn, sub, dm):
        # contraction split per head-group so the per-(token, group) rstd
        # lands as a per-partition scalar on the eviction; psA eviction on
        # scalar, psB fuse on vector, one output DMA per group
        psA = ps_tile([P, BLK])
        psB = ps_tile([P, BLK])
        for kk in (0, 1):
            nc.tensor.matmul(
                psA[:], ogT[:, kk, sub * 128:(sub + 1) * 128],
                wd_t[:, kk, dm * BLK:(dm + 1) * BLK],
                start=(kk == 0), stop=(kk == 1))
        for kk in (2, 3):
            nc.tensor.matmul(
                psB[:], ogT[:, kk, sub * 128:(sub + 1) * 128],
                wd_t[:, kk, dm * BLK:(dm + 1) * BLK],
                start=(kk == 2), stop=(kk == 3))
        rows = slice(nn * BLK + sub * 128, nn * BLK + (sub + 1) * 128)
        tmp = outsp.tile([P, BLK], F16, tag="ostmp")
        nc.scalar.activation(tmp[:], psA[:], AF.Copy,
                             scale=rstd_gn[:, sub, 0:1])
        osb = outsp.tile([P, BLK], F16, tag="osb")
        nc.vector.scalar_tensor_tensor(osb[:], psB[:],
                                       rstd_gn[:, sub, 1:2], tmp[:],
                                       AL.mult, AL.add)
        nc.sync.dma_start(d["outp"][rows, dm * BLK:(dm + 1) * BLK], osb[:])

    # prologue: block 0 projections emitted directly, with each sub's
    # norm+rope and XBAR transposes emitted right after its three groups
    # block-0 q/k projections k-outer across 8 PSUM banks: each weight
    # tile is consumed as it lands, so the PE keeps pace with the 8MB
    # weight DMA instead of stalling on per-group k-inner accumulation
    q_blk0 = [qkp.tile([P, DOUT], F16, tag="qb", name="qb") for _ in range(SUBS)]
    k_blk0 = [qkp.tile([P, DOUT], F16, tag="kb", name="kb") for _ in range(SUBS)]
    v_blk0 = [vp.tile([P, DOUT], F16, tag="vb", name="vb") for _ in range(SUBS)]
    ss_l0 = [ssp.tile([P, 8], F32, tag="ss", name="ss") for _ in range(SUBS)]
    cur = (q_blk0, k_blk0, v_blk0, ss_l0)
    ht0 = ht_blocks[0]
    ps_qk = [[ps_tile([P, DOUT]) for _ti in range(2)] for _s in range(SUBS)]
    for k in range(KT):
        for sub in range(SUBS):
            for ti in range(2):
                nc.tensor.matmul(
                    ps_qk[sub][ti][:], ht0[k][:, sub * 128:(sub + 1) * 128],
                    w_tiles[k][:, ti * DOUT:(ti + 1) * DOUT],
                    start=(k == 0), stop=(k == KT - 1))
    ccT0 = [alloc_chunk_T(), alloc_chunk_T()]
    for sub in range(SUBS):
        for ti, dest in ((0, q_blk0), (1, k_blk0)):
            sb = dest[sub]
            nc.scalar.copy(sb[:], ps_qk[sub][ti][:])
            sqs = sqscp.tile([P, DOUT], F16, tag="sqscratch")
            nc.vector.tensor_mul(sqs[:], sb[:], sb[:])
            nc.vector.tensor_reduce(
                ss_l0[sub][:, ti * 4:ti * 4 + 4],
                sqs[:].rearrange("p (h d) -> p h d", h=NHL),
                mybir.AxisListType.X, AL.add)
        emit_norm_rope(0, cur, subs_range=(sub,))
        emit_xpose(cur, ccT0, sub)
        # v projection (k-inner, weights resident by now) keeps the PE
        # busy while the vector engine norms/ropes
        ps_v = ps_tile([P, DOUT])
        for k in range(KT):
            nc.tensor.matmul(ps_v[:], ht0[k][:, sub * 128:(sub + 1) * 128],
                             w_tiles[k][:, 2 * DOUT:3 * DOUT],
                             start=(k == 0), stop=(k == KT - 1))
        nc.scalar.copy(v_blk0[sub][:], ps_v[:])
        if sub == 3:
            ht_blocks[1] = emit_ht_block(1, ht_big[1])
    ccT_cur = ccT0

    filler = []
    reserve = []  # dense groups held for the last block's attention phase
    st_cur = [st_c0]  # fp16 recurrent state, replaced each chunk

    def drain(k):
        for _ in range(min(k, len(filler))):
            filler.pop(0)()

    for n in range(NBLK):
        q_blk, k_blk, v_blk, ss_l = cur
        drain(len(filler))  # leftover dense from previous block

        ccT_nxt = None
        if n > 0:
            emit_norm_rope(n, cur)
            for sub in range(SUBS):
                emit_xpose(cur, ccT_cur, sub)

        # g projection emitted lazily (interleaved into attention)
        sig_blk = sigp.tile([P, NHL, BLK], F16, tag="sig")

        def g_proj_group(mg, ht=ht_blocks[n], sig=sig_blk):
            ps = ps_tile([P, BLK])
            for k in range(KT):
                nc.tensor.matmul(
                    ps[:],
                    w_tiles[k][:, 3 * DOUT + mg * 128:3 * DOUT + (mg + 1) * 128],
                    ht[k], start=(k == 0), stop=(k == KT - 1))
            nc.scalar.activation(sig[:, mg, :], ps[:], AF.Sigmoid)

        filler.extend([lambda mg=mg: g_proj_group(mg) for mg in range(NHL)])

        # enqueue next block's projection groups as filler
        if n + 1 < NBLK:
            groups, nxt = make_proj(ht_blocks[n + 1])
            filler.extend(groups)
        else:
            nxt = None
            filler.extend(reserve)
            reserve.clear()
        if n == 0:
            nc.sync.dma_start(wd_t[:], d["wd"])
            ht_blocks[2] = emit_ht_block(2, ht_big[2])
        drain(2)

        if DEBUG_TAPS and n == 0:
            for sub in range(SUBS):
                if "q" in TAP_SET:
                    nc.sync.dma_start(d["dbg_q"][:, sub, :], q_blk[sub][:])
                if "k" in TAP_SET:
                    nc.sync.dma_start(d["dbg_k"][:, sub, :], k_blk[sub][:])
                if "v" in TAP_SET:
                    nc.sync.dma_start(d["dbg_v"][:, sub, :], v_blk[sub][:])

        # ---- attention: 2 chunks of 256, head-pair batched ----
        ogT_blk = ogp.tile([P, NHL, BLK], F16, tag="ogT")
        rstd_gn = gnp.tile([P, SUBS, 2], F32, tag="grstd")
        for cc in range(2):
            subs = (2 * cc, 2 * cc + 1)
            qT_all, kT_all = ccT_cur[cc]

            st_c = st_cur[0]
            st_new = stcp.tile([P, NHL, HD], F16, tag="stc")
            st_lam = stlp.tile([P, NHL, HD], F16, tag="stlam")
            nc.vector.tensor_mul(
                st_lam[:], st_c[:],
                lamv_t[:].unsqueeze(2).to_broadcast((P, NHL, HD)))
            vd_c = []
            for si, sub in enumerate(subs):
                vd = smallp.tile([P, NHL, HD], F16, tag="vd")
                nc.vector.tensor_mul(
                    vd[:], v_blk[sub][:].rearrange("p (h e) -> p h e", h=NHL),
                    kdec_t[:, :, si].unsqueeze(2).to_broadcast((P, NHL, HD)))
                vd_c.append(vd)
            qdT_all = trp.tile([P, 2, NHL, 128], F16, tag="qdT")
            nc.vector.tensor_mul(qdT_all[:], qT_all[:], qdecb_t[:])
            drain(2)

            # per head-pair (= norm group): scores, o (intra+inter), square,
            # gate - shared PSUM banks, batched evictions
            sq_g = []
            for g in range(2):
                sT = []
                for hh in range(2):
                    h = 2 * g + hh
                    pst = ps_tile([P, 2 * CH])
                    for si in range(2):
                        nc.tensor.matmul(pst[:, si * CH:(si + 1) * CH],
                                         kT_all[:, si, h, :],
                                         qT_all[:, :, h, :],
                                         start=True, stop=True)
                    st = stp.tile([P, 2, CH], F16, tag="sT")
                    nc.vector.tensor_mul(
                        st[:], pst[:].rearrange("p (s c) -> p s c", s=2),
                        mask_t[:, h, :, :])
                    sT.append(st)
                drain(1)
                o_ps = ps_tile([P, 2, CH])
                for hh in range(2):
                    h = 2 * g + hh
                    for si, sub in enumerate(subs):
                        nc.tensor.matmul(
                            o_ps[:, hh, :],
                            v_blk[sub][:, h * HD:(h + 1) * HD],
                            sT[hh][:, si, :], start=(si == 0), stop=False)
                    nc.tensor.matmul(o_ps[:, hh, :], st_c[:, h, :],
                                     qdT_all[:, :, h, :],
                                     start=False, stop=True)
                # scale 1/64 before squaring: o can reach ~1e3 for
                # weak-decay heads and o^2 would overflow fp16
                sq = sqp.tile([P, 2, CH], F16, tag="sq")
                nc.scalar.activation(sq[:], o_ps[:], AF.Square,
                                     scale=1.0 / 64.0)
                nc.vector.tensor_mul(
                    ogT_blk[:, 2 * g:2 * g + 2, cc * CH:(cc + 1) * CH],
                    o_ps[:], sig_blk[:, 2 * g:2 * g + 2, cc * CH:(cc + 1) * CH])
                sq_g.append(sq)
                drain(1)

            # state update: all 4 heads share one PSUM bank; decay term
            # applied on the vector engine, fused into the eviction add
            dl_ps = ps_tile([P, NHL, HD])
            for h in range(NHL):
                for si, sub in enumerate(subs):
                    nc.tensor.matmul(
                        dl_ps[:, h, :], k_blk[sub][:, h * HD:(h + 1) * HD],
                        vd_c[si][:, h, :], start=(si == 0), stop=(si == 1))
                if h == 1:
                    drain(1)
            nc.vector.tensor_add(st_new[:], dl_ps[:], st_lam[:])
            st_cur[0] = st_new
            drain(1)

            # group norm rstd, batched [P, 4] = (si, g): head-pair sums
            # pre-added on vector, then sq (hd-major) x ones -> [tokens, 1]
            gsum = []
            for g in range(2):
                gs = sqp.tile([P, CH], F16, tag="gsum")
                nc.vector.tensor_add(gs[:], sq_g[g][:, 0, :],
                                     sq_g[g][:, 1, :])
                gsum.append(gs)
            gcol = ps_tile([P, 4])
            for si in range(2):
                for g in range(2):
                    nc.tensor.matmul(
                        gcol[:, si * 2 + g:si * 2 + g + 1],
                        gsum[g][:, si * 128:(si + 1) * 128],
                        ones_col[:], start=True, stop=True)
            grt = gnp.tile([P, 4], F32, tag="grt")
            nc.vector.tensor_scalar(grt[:], gcol[:], 4096.0 / (2 * HD), EPS,
                                    AL.mult, AL.add)
            nc.vector.reciprocal(grt[:], grt[:])
            nc.scalar.activation(
                rstd_gn[:, 2 * cc:2 * cc + 2, :].rearrange(
                    "p a b -> p (a b)"), grt[:], AF.Sqrt)
            drain(1)

            # interleave dense groups 1:1 into the remaining filler (proj
            # groups): a contiguous dense burst stalls on evictions
            dns = [lambda nn=n, og=ogT_blk, rs=rstd_gn, s=sub, m=dm:
                   dense_group(nn, og, rs, s, m)
                   for sub in (2 * cc, 2 * cc + 1)
                   for dm in range(HID // BLK)]
            if n == NBLK - 2:
                # hold back dense work to cover the last block's attention
                reserve.extend(dns[2:])
                dns = dns[:2]
            mixed = []
            while filler or dns:
                if filler:
                    mixed.append(filler.pop(0))
                if dns:
                    mixed.append(dns.pop(0))
            filler[:] = mixed
            drain(2)
            if n == 1 and cc == 0:
                # late prefetch of the last hT block into block-0's buffer
                # (sync ring; WAR on block-0 g-proj resolves early in this
                # block's attention)
                ht_blocks[3] = emit_ht_block(3, ht_big[0])

        if DEBUG_TAPS and n == 0:
            if "sg" in TAP_SET:
                nc.sync.dma_start(d["dbg_sg"], sig_blk[:])
            if "og" in TAP_SET:
                nc.sync.dma_start(d["dbg_og"], ogT_blk[:])
            if "st" in TAP_SET:
                st_dump = const.tile([P, NHL, HD], F32, tag="stdump")
                nc.vector.tensor_copy(st_dump[:], st_cur[0][:])
                nc.sync.dma_start(d["dbg_st"], st_dump[:])

        if nxt is not None:
            ccT_nxt = [alloc_chunk_T(), alloc_chunk_T()]
        cur = nxt
        ccT_cur = ccT_nxt

    drain(len(filler))


_NC_CACHE = None


def _get_module():
    global _NC_CACHE
    if _NC_CACHE is None:
        _NC_CACHE = _build_module()
    return _NC_CACHE


def _host_inputs(positions, hidden_states, recurrent_state, w_qkv, w_g,
                 w_dense, q_norm_w, k_norm_w, g_norm_w):
    """Build the 8 per-core input dicts."""
    F16NP = np.float16
    positions = np.asarray(positions)
    hidden_states = np.asarray(hidden_states, dtype=np.float32)
    recurrent_state = np.asarray(recurrent_state, dtype=np.float32)
    w_qkv = np.asarray(w_qkv, dtype=np.float32)
    w_g = np.asarray(w_g, dtype=np.float32)
    w_dense = np.asarray(w_dense, dtype=np.float32)
    q_norm_w = np.asarray(q_norm_w, dtype=np.float32)
    k_norm_w = np.asarray(k_norm_w, dtype=np.float32)
    g_norm_w = np.asarray(g_norm_w, dtype=np.float32)

    # rope tables from positions: [S, HALF] -> [128, S//128, HALF]
    inv_freq = 1.0 / (THETA ** (np.arange(HALF, dtype=np.float64) / HALF))
    ang = positions.astype(np.float64)[:, None] * inv_freq[None, :]
    cos = np.cos(ang).reshape(S // 128, 128, HALF).transpose(1, 0, 2)
    sin = np.sin(ang).reshape(S // 128, 128, HALF).transpose(1, 0, 2)
    cos = np.ascontiguousarray(cos.astype(F16NP))
    sin = np.ascontiguousarray(sin.astype(F16NP))

    qnw = np.ascontiguousarray(np.tile(q_norm_w[None, :], (128, 1))
                               .astype(F16NP))
    knw = np.ascontiguousarray(np.tile(k_norm_w[None, :], (128, 1))
                               .astype(F16NP))

    i_idx = np.arange(CH, dtype=np.float64)
    j_idx = np.arange(CH, dtype=np.float64)

    in_maps = []
    for core in range(NCORES):
        b = core // 4
        h0 = (core % 4) * NHL  # global head offset
        cs, ce = h0 * HD, (h0 + NHL) * HD

        hT = np.ascontiguousarray(hidden_states[b].T.astype(F16NP))

        w_all = np.concatenate(
            [w_qkv[:, cs:ce], w_qkv[:, NH * HD + cs:NH * HD + ce],
             w_qkv[:, 2 * NH * HD + cs:2 * NH * HD + ce],
             w_g[:, cs:ce]], axis=1)  # [HID, 4*DOUT]
        wqkvg = np.ascontiguousarray(
            w_all.reshape(KT, 128, 4 * DOUT).transpose(1, 0, 2).astype(F16NP))

        wd = (w_dense[cs:ce, :] * g_norm_w[cs:ce, None]).astype(F16NP)
        wd = np.ascontiguousarray(wd.reshape(NHL, 128, HID).transpose(1, 0, 2))

        loglam = _SLOPE_ALL[h0:h0 + NHL]  # [NHL] negative
        # maskt[p, h, jsub, i] = lam^(i - (jsub*128+p)) for i >= j else 0
        mm = np.where(i_idx[None, None, :] >= j_idx[None, :, None],
                      np.exp(loglam[:, None, None]
                             * (i_idx[None, None, :] - j_idx[None, :, None])),
                      0.0)  # [NHL, j, i]
        maskt = np.ascontiguousarray(
            mm.reshape(NHL, 2, 128, CH).transpose(2, 0, 1, 3).astype(F16NP))

        # qdec as a partition-broadcast table [p, si, h, t] = lam_h^(i+1),
        # i = si*128 + t; applied to qT ([d, si, h, t]) after the transpose
        qdec = np.exp(loglam[None, :, None] * (
            (i_idx.reshape(2, 1, 128)) + 1.0))  # [2, NHL, 128]
        qdecb = np.ascontiguousarray(
            np.broadcast_to(qdec[None].astype(F16NP), (128, 2, NHL, 128)))
        kdec = np.exp(loglam[:, None] * (CH - 1.0 - j_idx[None, :]))
        kdec = np.ascontiguousarray(
            kdec.reshape(NHL, 2, 128).transpose(2, 0, 1).astype(np.float32))
        lamv = np.ascontiguousarray(np.broadcast_to(
            np.exp(loglam * CH).astype(F16NP)[None, :], (128, NHL)))

        st0 = np.ascontiguousarray(
            recurrent_state[b, h0:h0 + NHL].transpose(1, 0, 2))  # [d, h, e]

        in_maps.append({
            "onec": np.ones((128, 1), F16NP),
            "hT": hT, "wqkvg": wqkvg, "wd": wd,
            "costab": cos, "sintab": sin, "qnw": qnw, "knw": knw,
            "maskt": maskt, "qdecb": qdecb, "kdec": kdec,
            "lamv": lamv,
            "st0": st0.astype(F16NP),
        })
    return in_maps


def kernel(**inputs):
    nc = _get_module()
    in_maps = _host_inputs(**inputs)
    res = run_bass_kernel_spmd(nc, in_maps, core_ids=list(range(NCORES)))
    outs = [r["outp"] for r in res.results]
    out = np.stack([outs[0] + outs[1] + outs[2] + outs[3],
                    outs[4] + outs[5] + outs[6] + outs[7]])
    return out.astype(np.float32)


# revision 16
# speedup vs baseline: 1.0229x; 1.0229x over previous
"""Trainium2 Bass kernel for BailingMoeV2.5 linear attention layer.

Sharding: 8 cores = 2 batches x 4 head-groups. Core c handles batch c//4,
heads 4*(c%4) .. +4 (of 16). Each core computes its head-slice of
qkv/g projections, chunked ALiBi-decayed linear attention, group-RMSNorm,
sigmoid gate, and a partial dense output (its 512 rows of w_dense).
Host sums the 4 partial outputs per batch.

All matmuls in fp16 (1 cyc/row on PE, 10-bit mantissa), fp32 PSUM
accumulation, fp32 recurrent state master.

Layout strategy per core:
  - hiddenT (fp16, [d_in, s]) host-pre-transposed; projections of q,k,v
    token-major (hiddenT tiles stationary), g head-dim-major (W_g stationary).
  - q,k norm+rope in token-major (free-dim reductions), then XBAR
    DMA-transposed per 128-token subtile into head-dim-major [d, si, h, t]
    tiles for the attention matmuls (no PE transposes, no PSUM traffic).
  - attention: per 256-chunk, per head-pair shared PSUM banks; o (intra +
    inter) accumulates [e, i]; group-norm rstd via PE ones-reduction
    batched [P, 4] per chunk; gate in head-dim-major.
  - dense: ogT (fp16) stationary, w_dense moving -> token-major partial
    out; eviction split scalar (psA*rstd0) || vector (psB*rstd1 + tmp),
    one 1KB-per-partition output DMA per group.
  - DMA rings: sync carries w/hT/outputs; scalar carries small consts and
    the XBAR transposes (parallel descriptor programming at startup).
"""

import math
from contextlib import ExitStack

import numpy as np

import concourse.mybir as mybir
import concourse.tile as tile
from concourse import bacc
from concourse.bass_utils import run_bass_kernel_spmd

dt = mybir.dt
F32 = dt.float32
F16 = dt.float16
AL = mybir.AluOpType
AF = mybir.ActivationFunctionType

# static model config
NH, HD, HID = 16, 128, 2048
ROT, HALF = 64, 32
EPS = 1e-6
THETA = 10000.0
LAYER_IDX, N_LAYERS = 1, 32
B, S = 2, 2048

DEBUG_TAPS = False
TAP_SET = ("q", "k", "v", "sg", "og", "st")

# kernel tiling config
NCORES = 8
NHL = 4            # heads per core
CH = 256           # internal chunk length (exact algebraic regrouping)
BLK = 512          # tokens per projection block
NBLK = S // BLK    # 4
SUBS = BLK // 128  # 4 s-subtiles per block
KT = HID // 128    # 16 d_in tiles
DOUT = NHL * HD    # 512 per tensor (q,k,v,g)


def _base_slopes(n):
    start = 2 ** (-(2 ** (-(math.log2(n) - 3))))
    return [start * (start ** i) for i in range(n)]


_SLOPE_ALL = -np.array(_base_slopes(NH), dtype=np.float64) * (
    1.0 - (LAYER_IDX - 1) / (N_LAYERS - 1) + 1e-5
)  # [NH] negative log-decay


def _build_module():
    nc = bacc.Bacc("TRN2", target_bir_lowering=False, debug=False,
                   num_devices=NCORES)

    f16in = lambda name, shape: nc.dram_tensor(
        name, shape, F16, kind="ExternalInput").ap()
    f32in = lambda name, shape: nc.dram_tensor(
        name, shape, F32, kind="ExternalInput").ap()

    d = {
        "hT": f16in("hT", [HID, S]),
        "wqkvg": f16in("wqkvg", [128, KT, 4 * DOUT]),
        "wd": f16in("wd", [128, NHL, HID]),
        "costab": f16in("costab", [128, S // 128, HALF]),
        "sintab": f16in("sintab", [128, S // 128, HALF]),
        "qnw": f16in("qnw", [128, HD]),
        "knw": f16in("knw", [128, HD]),
        "maskt": f16in("maskt", [128, NHL, 2, CH]),
        "onec": f16in("onec", [128, 1]),
        "qdecb": f16in("qdecb", [128, 2, NHL, 128]),
        "kdec": f32in("kdec", [128, NHL, 2]),
        "lamv": f16in("lamv", [128, NHL]),
        "st0": f16in("st0", [128, NHL, HD]),
        "outp": nc.dram_tensor("outp", [S, HID], F16,
                               kind="ExternalOutput").ap(),
    }
    if DEBUG_TAPS:
        for nm, shape, dtp in [("dbg_q", [128, SUBS, DOUT], F16),
                               ("dbg_k", [128, SUBS, DOUT], F16),
                               ("dbg_v", [128, SUBS, DOUT], F16),
                               ("dbg_og", [128, NHL, BLK], F16),
                               ("dbg_st", [128, NHL, HD], F32),
                               ("dbg_sg", [128, NHL, BLK], F16)]:
            d[nm] = nc.dram_tensor(nm, shape, dtp,
                                   kind="ExternalOutput").ap()

    with tile.TileContext(nc) as tc, ExitStack() as ctx, \
            nc.allow_low_precision(reason="fp16 operands, fp32 accumulate"):
        _body(nc, tc, ctx, d)

    nc.compile()
    return nc


def _body(nc, tc, ctx, d):
    P = 128

    pool = lambda name, bufs: ctx.enter_context(
        tc.tile_pool(name=name, bufs=bufs))
    const = pool("const", 1)      # tables, masks, state (~11k)
    wpool = pool("wpool", 1)      # 80k: resident weights (fp16)
    htpool = pool("ht", 3)        # 48k: hT big tiles, 3 blocks resident
    qkp = pool("qkp", 5)          # 10k: q/k token-major
    vp = pool("vp", 8)            # 8k: v token-major, 2 blocks
    sigp = pool("sigp", 2)        # 8k: sigmoid(g) head-dim-major (fp16)
    sqscp = pool("sqsc", 1)       # 1k: sumsq squares scratch (fp16)
    ropep = pool("ropep", 1)      # 1k: rope m1..m4 (fp16)
    ssp = pool("ssp", 6)          # ~0.5k: sumsq/rstd chains (fp32)
    trp = pool("trp", 2)          # 12k: qT/kT/qdT chunk tiles (fp16)
    stp = pool("stp", 2)          # 2k: masked scoresT (fp16)
    smallp = pool("smallp", 3)    # 3k: kdec-scaled v (fp16)
    stcp = pool("stcp", 2)        # 2k: fp16 state snapshot
    stlp = pool("stlp", 2)        # 2k: decayed state (fp16)
    sqp = pool("sqp", 3)          # 3k: o squares (2 alive per chunk)
    gnp = pool("gnp", 2)          # <1k: group-norm rstd chain
    ogp = pool("ogp", 2)          # 8k: ogT fp16 block
    outsp = pool("outs", 3)       # 6k: dense output staging (fp16)

    psum = ctx.enter_context(tc.tile_pool(name="ps", bufs=8, space="PSUM"))
    psn = [0]

    def ps_tile(shape, dtype=F32):
        psn[0] += 1
        return psum.tile(shape, dtype, tag="ps", name=f"ps{psn[0]}")

    def loadc(name, shape, dtype=F16):
        t = const.tile(shape, dtype, tag=name, name=name)
        nc.scalar.dma_start(t[:], d[name])
        return t

    # HAM warmup: real matmuls on a zeroed scratch tile (no DMA dependency)
    # keep the PE busy during the initial weight DMA so the clock-gate opens
    # (1.2 -> 2.4 GHz) before the first real matmul
    scratch = const.tile([P, 256], F16, tag="scratch", name="scratch")
    nc.vector.memset(scratch[:], 0.0)
    wrm1 = ps_tile([P, 256])
    wrm2 = ps_tile([P, 256])
    for i in range(8):
        nc.tensor.matmul(wrm1[:], scratch[:, 0:128], scratch[:],
                         start=(i == 0), stop=(i == 7))
        nc.tensor.matmul(wrm2[:], scratch[:, 0:128], scratch[:],
                         start=(i == 0), stop=(i == 7))

    # big hT tiles: block 0 filled by per-k DMAs interleaved with the weight
    # stream (fine-grained deps for the prologue); blocks 1,2 one-shot
    ht_big = [htpool.tile([P, KT, BLK], F16, tag="htb", name=f"htb{i}")
              for i in range(3)]
    w_tiles = []
    for k in range(KT):
        wt = wpool.tile([P, 4 * DOUT], F16, tag=f"w{k}", name=f"w{k}")
        nc.sync.dma_start(wt[:], d["wqkvg"][:, k, :])
        w_tiles.append(wt)
        nc.sync.dma_start(ht_big[0][:, k, :],
                          d["hT"][k * 128:(k + 1) * 128, 0:BLK])
    # small consts on the scalar ring - parallel descriptor programming,
    # all landed before the first norm/attention consumers
    cos_t = loadc("costab", [P, S // 128, HALF])
    sin_t = loadc("sintab", [P, S // 128, HALF])
    qnw_t = loadc("qnw", [P, HD])
    knw_t = loadc("knw", [P, HD])
    mask_t = loadc("maskt", [P, NHL, 2, CH])
    qdecb_t = loadc("qdecb", [P, 2, NHL, 128])
    kdec_t = loadc("kdec", [P, NHL, 2], F32)
    lamv_t = loadc("lamv", [P, NHL])
    st_c0 = loadc("st0", [P, NHL, HD])
    ones_col = loadc("onec", [P, 1])
    # bulk prefetch of later hT blocks on the sync ring
    def emit_ht_block(nn, buf):
        nc.sync.dma_start(
            buf[:], d["hT"].rearrange("(k p) s -> p k s", p=128)
            [:, :, nn * BLK:(nn + 1) * BLK])
        return [buf[:, k, :] for k in range(KT)]

    ht_blocks = [None] * NBLK
    ht_blocks[0] = [ht_big[0][:, k, :] for k in range(KT)]
    wd_t = wpool.tile([P, NHL, HID], F16, tag="wd", name="wd")

    def make_proj(ht):
        """Allocate a block's q/k/v tiles; return 12 emit-closures (one PSUM
        accumulation group each: 16 matmuls + eviction + q/k sumsq)."""
        q_blk = [qkp.tile([P, DOUT], F16, tag="qb", name="qb") for _ in range(SUBS)]
        k_blk = [qkp.tile([P, DOUT], F16, tag="kb", name="kb") for _ in range(SUBS)]
        v_blk = [vp.tile([P, DOUT], F16, tag="vb", name="vb") for _ in range(SUBS)]
        ss_l = [ssp.tile([P, 8], F32, tag="ss", name="ss") for _ in range(SUBS)]
        groups = []
        sqg = []
        for sub in range(SUBS):
            for ti, dest in enumerate((q_blk, k_blk, v_blk)):
                def grp(sub=sub, ti=ti, dest=dest, ht=ht):
                    ps = ps_tile([P, DOUT])
                    for k in range(KT):
                        nc.tensor.matmul(
                            ps[:], ht[k][:, sub * 128:(sub + 1) * 128],
                            w_tiles[k][:, ti * DOUT:(ti + 1) * DOUT],
                            start=(k == 0), stop=(k == KT - 1))
                    sb = dest[sub]
                    nc.scalar.copy(sb[:], ps[:])
                groups.append(grp)
                if ti < 2:
                    # sumsq deferred to the end of the filler list: these
                    # vector ops are only needed at the next block's norm,
                    # and emitting them late keeps the vector queue clear
                    # for attention-critical masks/gates
                    def sq(sub=sub, ti=ti, dest=dest, ss_t=ss_l[sub]):
                        sb = dest[sub]
                        sqs = sqscp.tile([P, DOUT], F16, tag="sqscratch")
                        nc.vector.tensor_mul(sqs[:], sb[:], sb[:])
                        nc.vector.tensor_reduce(
                            ss_t[:, ti * 4:ti * 4 + 4],
                            sqs[:].rearrange("p (h d) -> p h d", h=NHL),
                            mybir.AxisListType.X, AL.add)
                    sqg.append(sq)
        return groups + sqg, (q_blk, k_blk, v_blk, ss_l)

    def emit_norm_rope(n, blk_state, subs_range=None):
        q_blk, k_blk, v_blk, ss_l = blk_state
        for sub in (range(SUBS) if subs_range is None else subs_range):
            gs = n * SUBS + sub
            rtmp = ssp.tile([P, 8], F32, tag="rstdt")
            nc.vector.tensor_scalar(rtmp[:], ss_l[sub][:], 1.0 / HD, EPS,
                                    AL.mult, AL.add)
            nc.vector.reciprocal(rtmp[:], rtmp[:])
            rstd_t = ssp.tile([P, 8], F16, tag="rstd")
            nc.scalar.activation(rstd_t[:], rtmp[:], AF.Sqrt)
            for ti, (blk, nw_t) in enumerate(((q_blk, qnw_t), (k_blk, knw_t))):
                x = blk[sub]
                x3 = x.rearrange("p (h d) -> p h d", h=NHL)
                rsl = rstd_t[:, ti * 4:ti * 4 + 4]
                nc.vector.tensor_mul(
                    x3, x3, rsl.unsqueeze(2).to_broadcast((P, NHL, HD)))
                nc.vector.tensor_mul(
                    x3, x3, nw_t[:].unsqueeze(1).to_broadcast((P, NHL, HD)))
                x1, x2 = x3[:, :, 0:HALF], x3[:, :, HALF:ROT]
                cosb = cos_t[:, gs, :].unsqueeze(1).to_broadcast(
                    (P, NHL, HALF))
                sinb = sin_t[:, gs, :].unsqueeze(1).to_broadcast(
                    (P, NHL, HALF))
                m1 = ropep.tile([P, NHL, HALF], F16, tag="m1")
                m2 = ropep.tile([P, NHL, HALF], F16, tag="m2")
                m3 = ropep.tile([P, NHL, HALF], F16, tag="m3")
                m4 = ropep.tile([P, NHL, HALF], F16, tag="m4")
                nc.vector.tensor_mul(m1[:], x1, cosb)
                nc.vector.tensor_mul(m2[:], x2, sinb)
                nc.vector.tensor_mul(m3[:], x2, cosb)
                nc.vector.tensor_mul(m4[:], x1, sinb)
                nc.vector.tensor_sub(x1, m1[:], m2[:])
                nc.vector.tensor_add(x2, m3[:], m4[:])

    def alloc_chunk_T():
        # [d, si, h, t] head-dim-major chunk tiles, filled by XBAR transposes
        qT = trp.tile([P, 2, NHL, 128], F16, tag="qT", name="qT")
        kT = trp.tile([P, 2, NHL, 128], F16, tag="kT", name="kT")
        return qT, kT

    def emit_xpose(blk_state, cc_tiles, sub):
        # XBAR DMA transpose (scalar ring): q_blk[sub] [t,(h d)] -> [d,h,t]
        q_blk, k_blk, v_blk, ss_l = blk_state
        qT, kT = cc_tiles[sub // 2]
        si = sub % 2
        nc.sync.dma_start_transpose(qT[:, si], q_blk[sub][:])
        nc.scalar.dma_start_transpose(kT[:, si], k_blk[sub][:])

    def dense_group(nn, ogT, rstd_gn, sub, dm):
        # contraction split per head-group so the per-(token, group) rstd
        # lands as a per-partition scalar on the eviction; psA eviction on
        # scalar, psB fuse on vector, one output DMA per group
        psA = ps_tile([P, BLK])
        psB = ps_tile([P, BLK])
        for kk in (0, 1):
            nc.tensor.matmul(
                psA[:], ogT[:, kk, sub * 128:(sub + 1) * 128],
                wd_t[:, kk, dm * BLK:(dm + 1) * BLK],
                start=(kk == 0), stop=(kk == 1))
        for kk in (2, 3):
            nc.tensor.matmul(
                psB[:], ogT[:, kk, sub * 128:(sub + 1) * 128],
                wd_t[:, kk, dm * BLK:(dm + 1) * BLK],
                start=(kk == 2), stop=(kk == 3))
        rows = slice(nn * BLK + sub * 128, nn * BLK + (sub + 1) * 128)
        tmp = outsp.tile([P, BLK], F16, tag="ostmp")
        nc.scalar.activation(tmp[:], psA[:], AF.Copy,
                             scale=rstd_gn[:, sub, 0:1])
        osb = outsp.tile([P, BLK], F16, tag="osb")
        nc.vector.scalar_tensor_tensor(osb[:], psB[:],
                                       rstd_gn[:, sub, 1:2], tmp[:],
                                       AL.mult, AL.add)
        nc.sync.dma_start(d["outp"][rows, dm * BLK:(dm + 1) * BLK], osb[:])

    # prologue: block 0 projections emitted directly, with each sub's
    # norm+rope and XBAR transposes emitted right after its three groups
    # block-0 q/k projections k-outer across 8 PSUM banks: each weight
    # tile is consumed as it lands, so the PE keeps pace with the 8MB
    # weight DMA instead of stalling on per-group k-inner accumulation
    q_blk0 = [qkp.tile([P, DOUT], F16, tag="qb", name="qb") for _ in range(SUBS)]
    k_blk0 = [qkp.tile([P, DOUT], F16, tag="kb", name="kb") for _ in range(SUBS)]
    v_blk0 = [vp.tile([P, DOUT], F16, tag="vb", name="vb") for _ in range(SUBS)]
    ss_l0 = [ssp.tile([P, 8], F32, tag="ss", name="ss") for _ in range(SUBS)]
    cur = (q_blk0, k_blk0, v_blk0, ss_l0)
    ht0 = ht_blocks[0]
    ps_qk = [[ps_tile([P, DOUT]) for _ti in range(2)] for _s in range(SUBS)]
    for k in range(KT):
        for sub in range(SUBS):
            for ti in range(2):
                nc.tensor.matmul(
                    ps_qk[sub][ti][:], ht0[k][:, sub * 128:(sub + 1) * 128],
                    w_tiles[k][:, ti * DOUT:(ti + 1) * DOUT],
                    start=(k == 0), stop=(k == KT - 1))
    ccT0 = [alloc_chunk_T(), alloc_chunk_T()]
    for sub in range(SUBS):
        for ti, dest in ((0, q_blk0), (1, k_blk0)):
            sb = dest[sub]
            nc.scalar.copy(sb[:], ps_qk[sub][ti][:])
            sqs = sqscp.tile([P, DOUT], F16, tag="sqscratch")
            nc.vector.tensor_mul(sqs[:], sb[:], sb[:])
            nc.vector.tensor_reduce(
                ss_l0[sub][:, ti * 4:ti * 4 + 4],
                sqs[:].rearrange("p (h d) -> p h d", h=NHL),
                mybir.AxisListType.X, AL.add)
        emit_norm_rope(0, cur, subs_range=(sub,))
        emit_xpose(cur, ccT0, sub)
        # v projection (k-inner, weights resident by now) keeps the PE
        # busy while the vector engine norms/ropes
        ps_v = ps_tile([P, DOUT])
        for k in range(KT):
            nc.tensor.matmul(ps_v[:], ht0[k][:, sub * 128:(sub + 1) * 128],
                             w_tiles[k][:, 2 * DOUT:3 * DOUT],
                             start=(k == 0), stop=(k == KT - 1))
        nc.scalar.copy(v_blk0[sub][:], ps_v[:])
        if sub == 3:
            ht_blocks[1] = emit_ht_block(1, ht_big[1])
    ccT_cur = ccT0

    filler = []
    reserve = []  # dense groups held for the last block's attention phase
    st_cur = [st_c0]  # fp16 recurrent state, replaced each chunk

    def drain(k):
        for _ in range(min(k, len(filler))):
            filler.pop(0)()

    for n in range(NBLK):
        q_blk, k_blk, v_blk, ss_l = cur
        drain(len(filler))  # leftover dense from previous block

        ccT_nxt = None
        if n > 0:
            emit_norm_rope(n, cur)
            for sub in range(SUBS):
                emit_xpose(cur, ccT_cur, sub)

        # g projection emitted lazily (interleaved into attention)
        sig_blk = sigp.tile([P, NHL, BLK], F16, tag="sig")

        def g_proj_group(mg, ht=ht_blocks[n], sig=sig_blk):
            ps = ps_tile([P, BLK])
            for k in range(KT):
                nc.tensor.matmul(
                    ps[:],
                    w_tiles[k][:, 3 * DOUT + mg * 128:3 * DOUT + (mg + 1) * 128],
                    ht[k], start=(k == 0), stop=(k == KT - 1))
            nc.scalar.activation(sig[:, mg, :], ps[:], AF.Sigmoid)

        filler.extend([lambda mg=mg: g_proj_group(mg) for mg in range(NHL)])

        # enqueue next block's projection groups as filler
        if n + 1 < NBLK:
            groups, nxt = make_proj(ht_blocks[n + 1])
            filler.extend(groups)
        else:
            nxt = None
            filler.extend(reserve)
            reserve.clear()
        if n == 0:
            nc.sync.dma_start(wd_t[:], d["wd"])
            ht_blocks[2] = emit_ht_block(2, ht_big[2])
        drain(2)

        if DEBUG_TAPS and n == 0:
            for sub in range(SUBS):
                if "q" in TAP_SET:
                    nc.sync.dma_start(d["dbg_q"][:, sub, :], q_blk[sub][:])
                if "k" in TAP_SET:
                    nc.sync.dma_start(d["dbg_k"][:, sub, :], k_blk[sub][:])
                if "v" in TAP_SET:
                    nc.sync.dma_start(d["dbg_v"][:, sub, :], v_blk[sub][:])

        # ---- attention: 2 chunks of 256, head-pair batched ----
        ogT_blk = ogp.tile([P, NHL, BLK], F16, tag="ogT")
        rstd_gn = gnp.tile([P, SUBS, 2], F32, tag="grstd")
        for cc in range(2):
            subs = (2 * cc, 2 * cc + 1)
            qT_all, kT_all = ccT_cur[cc]

            st_c = st_cur[0]
            st_new = stcp.tile([P, NHL, HD], F16, tag="stc")
            st_lam = stlp.tile([P, NHL, HD], F16, tag="stlam")
            nc.vector.tensor_mul(
                st_lam[:], st_c[:],
                lamv_t[:].unsqueeze(2).to_broadcast((P, NHL, HD)))
            vd_c = []
            for si, sub in enumerate(subs):
                vd = smallp.tile([P, NHL, HD], F16, tag="vd")
                nc.vector.tensor_mul(
                    vd[:], v_blk[sub][:].rearrange("p (h e) -> p h e", h=NHL),
                    kdec_t[:, :, si].unsqueeze(2).to_broadcast((P, NHL, HD)))
                vd_c.append(vd)
            qdT_all = trp.tile([P, 2, NHL, 128], F16, tag="qdT")
            nc.vector.tensor_mul(qdT_all[:], qT_all[:], qdecb_t[:])
            drain(2)

            # per head-pair (= norm group): scores, o (intra+inter), square,
            # gate - shared PSUM banks, batched evictions
            sq_g = []
            for g in range(2):
                sT = []
                for hh in range(2):
                    h = 2 * g + hh
                    pst = ps_tile([P, 2 * CH])
                    for si in range(2):
                        nc.tensor.matmul(pst[:, si * CH:(si + 1) * CH],
                                         kT_all[:, si, h, :],
                                         qT_all[:, :, h, :],
                                         start=True, stop=True)
                    st = stp.tile([P, 2, CH], F16, tag="sT")
                    nc.vector.tensor_mul(
                        st[:], pst[:].rearrange("p (s c) -> p s c", s=2),
                        mask_t[:, h, :, :])
                    sT.append(st)
                drain(1)
                o_ps = ps_tile([P, 2, CH])
                for hh in range(2):
                    h = 2 * g + hh
                    for si, sub in enumerate(subs):
                        nc.tensor.matmul(
                            o_ps[:, hh, :],
                            v_blk[sub][:, h * HD:(h + 1) * HD],
                            sT[hh][:, si, :], start=(si == 0), stop=False)
                    nc.tensor.matmul(o_ps[:, hh, :], st_c[:, h, :],
                                     qdT_all[:, :, h, :],
                                     start=False, stop=True)
                # scale 1/64 before squaring: o can reach ~1e3 for
                # weak-decay heads and o^2 would overflow fp16
                sq = sqp.tile([P, 2, CH], F16, tag="sq")
                nc.scalar.activation(sq[:], o_ps[:], AF.Square,
                                     scale=1.0 / 64.0)
                nc.vector.tensor_mul(
                    ogT_blk[:, 2 * g:2 * g + 2, cc * CH:(cc + 1) * CH],
                    o_ps[:], sig_blk[:, 2 * g:2 * g + 2, cc * CH:(cc + 1) * CH])
                sq_g.append(sq)
                drain(1)

            # state update: all 4 heads share one PSUM bank; decay term
            # applied on the vector engine, fused into the eviction add
            dl_ps = ps_tile([P, NHL, HD])
            for h in range(NHL):
                for si, sub in enumerate(subs):
                    nc.tensor.matmul(
                        dl_ps[:, h, :], k_blk[sub][:, h * HD:(h + 1) * HD],
                        vd_c[si][:, h, :], start=(si == 0), stop=(si == 1))
                if h == 1:
                    drain(1)
            nc.vector.tensor_add(st_new[:], dl_ps[:], st_lam[:])
            st_cur[0] = st_new
            drain(1)

            # group norm rstd, batched [P, 4] = (si, g): head-pair sums
            # pre-added on vector, then sq (hd-major) x ones -> [tokens, 1]
            gsum = []
            for g in range(2):
                gs = sqp.tile([P, CH], F16, tag="gsum")
                nc.vector.tensor_add(gs[:], sq_g[g][:, 0, :],
                                     sq_g[g][:, 1, :])
                gsum.append(gs)
            gcol = ps_tile([P, 4])
            for si in range(2):
                for g in range(2):
                    nc.tensor.matmul(
                        gcol[:, si * 2 + g:si * 2 + g + 1],
                        gsum[g][:, si * 128:(si + 1) * 128],
                        ones_col[:], start=True, stop=True)
            grt = gnp.tile([P, 4], F32, tag="grt")
            nc.vector.tensor_scalar(grt[:], gcol[:], 4096.0 / (2 * HD), EPS,
                                    AL.mult, AL.add)
            nc.vector.reciprocal(grt[:], grt[:])
            nc.scalar.activation(
                rstd_gn[:, 2 * cc:2 * cc + 2, :].rearrange(
                    "p a b -> p (a b)"), grt[:], AF.Sqrt)
            drain(1)

            # interleave dense groups 1:1 into the remaining filler (proj
            # groups): a contiguous dense burst stalls on evictions
            dns = [lambda nn=n, og=ogT_blk, rs=rstd_gn, s=sub, m=dm:
                   dense_group(nn, og, rs, s, m)
                   for sub in (2 * cc, 2 * cc + 1)
                   for dm in range(HID // BLK)]
            if n == NBLK - 2:
                # hold back dense work to cover the last block's attention
                reserve.extend(dns[2:])
                dns = dns[:2]
            mixed = []
            while filler or dns:
                if filler:
                    mixed.append(filler.pop(0))
                if dns:
                    mixed.append(dns.pop(0))
            filler[:] = mixed
            drain(2)
            if n == 1 and cc == 0:
                # late prefetch of the last hT block into block-0's buffer
                # (sync ring; WAR on block-0 g-proj resolves early in this
                # block's attention)
                ht_blocks[3] = emit_ht_block(3, ht_big[0])

        if DEBUG_TAPS and n == 0:
            if "sg" in TAP_SET:
                nc.sync.dma_start(d["dbg_sg"], sig_blk[:])
            if "og" in TAP_SET:
                nc.sync.dma_start(d["dbg_og"], ogT_blk[:])
            if "st" in TAP_SET:
                st_dump = const.tile([P, NHL, HD], F32, tag="stdump")
                nc.vector.tensor_copy(st_dump[:], st_cur[0][:])
                nc.sync.dma_start(d["dbg_st"], st_dump[:])

        if nxt is not None:
            ccT_nxt = [alloc_chunk_T(), alloc_chunk_T()]
        cur = nxt
        ccT_cur = ccT_nxt

    drain(len(filler))


_NC_CACHE = None


def _get_module():
    global _NC_CACHE
    if _NC_CACHE is None:
        _NC_CACHE = _build_module()
    return _NC_CACHE


def _host_inputs(positions, hidden_states, recurrent_state, w_qkv, w_g,
                 w_dense, q_norm_w, k_norm_w, g_norm_w):
    """Build the 8 per-core input dicts."""
    F16NP = np.float16
    positions = np.asarray(positions)
    hidden_states = np.asarray(hidden_states, dtype=np.float32)
    recurrent_state = np.asarray(recurrent_state, dtype=np.float32)
    w_qkv = np.asarray(w_qkv, dtype=np.float32)
    w_g = np.asarray(w_g, dtype=np.float32)
    w_dense = np.asarray(w_dense, dtype=np.float32)
    q_norm_w = np.asarray(q_norm_w, dtype=np.float32)
    k_norm_w = np.asarray(k_norm_w, dtype=np.float32)
    g_norm_w = np.asarray(g_norm_w, dtype=np.float32)

    # rope tables from positions: [S, HALF] -> [128, S//128, HALF]
    inv_freq = 1.0 / (THETA ** (np.arange(HALF, dtype=np.float64) / HALF))
    ang = positions.astype(np.float64)[:, None] * inv_freq[None, :]
    cos = np.cos(ang).reshape(S // 128, 128, HALF).transpose(1, 0, 2)
    sin = np.sin(ang).reshape(S // 128, 128, HALF).transpose(1, 0, 2)
    cos = np.ascontiguousarray(cos.astype(F16NP))
    sin = np.ascontiguousarray(sin.astype(F16NP))

    qnw = np.ascontiguousarray(np.tile(q_norm_w[None, :], (128, 1))
                               .astype(F16NP))
    knw = np.ascontiguousarray(np.tile(k_norm_w[None, :], (128, 1))
                               .astype(F16NP))

    i_idx = np.arange(CH, dtype=np.float64)
    j_idx = np.arange(CH, dtype=np.float64)

    in_maps = []
    for core in range(NCORES):
        b = core // 4
        h0 = (core % 4) * NHL  # global head offset
        cs, ce = h0 * HD, (h0 + NHL) * HD

        hT = np.ascontiguousarray(hidden_states[b].T.astype(F16NP))

        w_all = np.concatenate(
            [w_qkv[:, cs:ce], w_qkv[:, NH * HD + cs:NH * HD + ce],
             w_qkv[:, 2 * NH * HD + cs:2 * NH * HD + ce],
             w_g[:, cs:ce]], axis=1)  # [HID, 4*DOUT]
        wqkvg = np.ascontiguousarray(
            w_all.reshape(KT, 128, 4 * DOUT).transpose(1, 0, 2).astype(F16NP))

        wd = (w_dense[cs:ce, :] * g_norm_w[cs:ce, None]).astype(F16NP)
        wd = np.ascontiguousarray(wd.reshape(NHL, 128, HID).transpose(1, 0, 2))

        loglam = _SLOPE_ALL[h0:h0 + NHL]  # [NHL] negative
        # maskt[p, h, jsub, i] = lam^(i - (jsub*128+p)) for i >= j else 0
        mm = np.where(i_idx[None, None, :] >= j_idx[None, :, None],
                      np.exp(loglam[:, None, None]
                             * (i_idx[None, None, :] - j_idx[None, :, None])),
                      0.0)  # [NHL, j, i]
        maskt = np.ascontiguousarray(
            mm.reshape(NHL, 2, 128, CH).transpose(2, 0, 1, 3).astype(F16NP))

        # qdec as a partition-broadcast table [p, si, h, t] = lam_h^(i+1),
        # i = si*128 + t; applied to qT ([d, si, h, t]) after the transpose
        qdec = np.exp(loglam[None, :, None] * (
            (i_idx.reshape(2, 1, 128)) + 1.0))  # [2, NHL, 128]
        qdecb = np.ascontiguousarray(
            np.broadcast_to(qdec[None].astype(F16NP), (128, 2, NHL, 128)))
        kdec = np.exp(loglam[:, None] * (CH - 1.0 - j_idx[None, :]))
        kdec = np.ascontiguousarray(
            kdec.reshape(NHL, 2, 128).transpose(2, 0, 1).astype(np.float32))
        lamv = np.ascontiguousarray(np.broadcast_to(
            np.exp(loglam * CH).astype(F16NP)[None, :], (128, NHL)))

        st0 = np.ascontiguousarray(
            recurrent_state[b, h0:h0 + NHL].transpose(1, 0, 2))  # [d, h, e]

        in_maps.append({
            "onec": np.ones((128, 1), F16NP),
            "hT": hT, "wqkvg": wqkvg, "wd": wd,
            "costab": cos, "sintab": sin, "qnw": qnw, "knw": knw,
            "maskt": maskt, "qdecb": qdecb, "kdec": kdec,
            "lamv": lamv,
            "st0": st0.astype(F16NP),
        })
    return in_maps


def kernel(**inputs):
    nc = _get_module()
    in_maps = _host_inputs(**inputs)
    res = run_bass_kernel_spmd(nc, in_maps, core_ids=list(range(NCORES)))
    outs = [r["outp"] for r in res.results]
    out = np.stack([outs[0] + outs[1] + outs[2] + outs[3],
                    outs[4] + outs[5] + outs[6] + outs[7]])
    return out.astype(np.float32)


# revision 17
# speedup vs baseline: 1.0359x; 1.0127x over previous
"""Trainium2 Bass kernel for BailingMoeV2.5 linear attention layer.

Sharding: 8 cores = 2 batches x 4 head-groups. Core c handles batch c//4,
heads 4*(c%4) .. +4 (of 16). Each core computes its head-slice of
qkv/g projections, chunked ALiBi-decayed linear attention, group-RMSNorm,
sigmoid gate, and a partial dense output (its 512 rows of w_dense).
Host sums the 4 partial outputs per batch.

All matmuls in fp16 (1 cyc/row on PE, 10-bit mantissa), fp32 PSUM
accumulation, fp32 recurrent state master.

Layout strategy per core:
  - hiddenT (fp16, [d_in, s]) host-pre-transposed; projections of q,k,v
    token-major (hiddenT tiles stationary), g head-dim-major (W_g stationary).
  - q,k norm+rope in token-major (free-dim reductions), then XBAR
    DMA-transposed per 128-token subtile into head-dim-major [d, si, h, t]
    tiles for the attention matmuls (no PE transposes, no PSUM traffic).
  - attention: per 256-chunk, per head-pair shared PSUM banks; o (intra +
    inter) accumulates [e, i]; group-norm rstd via PE ones-reduction
    batched [P, 4] per chunk; gate in head-dim-major.
  - dense: ogT (fp16) stationary, w_dense moving -> token-major partial
    out; eviction split scalar (psA*rstd0) || vector (psB*rstd1 + tmp),
    one 1KB-per-partition output DMA per group.
  - DMA rings: sync carries w/hT/outputs; scalar carries small consts and
    the XBAR transposes (parallel descriptor programming at startup).
"""

import math
from contextlib import ExitStack

import numpy as np

import concourse.mybir as mybir
import concourse.tile as tile
from concourse import bacc
from concourse.bass_utils import run_bass_kernel_spmd

dt = mybir.dt
F32 = dt.float32
F16 = dt.float16
AL = mybir.AluOpType
AF = mybir.ActivationFunctionType

# static model config
NH, HD, HID = 16, 128, 2048
ROT, HALF = 64, 32
EPS = 1e-6
THETA = 10000.0
LAYER_IDX, N_LAYERS = 1, 32
B, S = 2, 2048

DEBUG_TAPS = False
TAP_SET = ("q", "k", "v", "sg", "og", "st")

# kernel tiling config
NCORES = 8
NHL = 4            # heads per core
CH = 256           # internal chunk length (exact algebraic regrouping)
BLK = 512          # tokens per projection block
NBLK = S // BLK    # 4
SUBS = BLK // 128  # 4 s-subtiles per block
KT = HID // 128    # 16 d_in tiles
DOUT = NHL * HD    # 512 per tensor (q,k,v,g)


def _base_slopes(n):
    start = 2 ** (-(2 ** (-(math.log2(n) - 3))))
    return [start * (start ** i) for i in range(n)]


_SLOPE_ALL = -np.array(_base_slopes(NH), dtype=np.float64) * (
    1.0 - (LAYER_IDX - 1) / (N_LAYERS - 1) + 1e-5
)  # [NH] negative log-decay


def _build_module():
    nc = bacc.Bacc("TRN2", target_bir_lowering=False, debug=False,
                   num_devices=NCORES)

    f16in = lambda name, shape: nc.dram_tensor(
        name, shape, F16, kind="ExternalInput").ap()
    f32in = lambda name, shape: nc.dram_tensor(
        name, shape, F32, kind="ExternalInput").ap()

    d = {
        "hT": f16in("hT", [HID, S]),
        "wqkvg": f16in("wqkvg", [128, KT, 4 * DOUT]),
        "wd": f16in("wd", [128, NHL, HID]),
        "costab": f16in("costab", [128, S // 128, HALF]),
        "sintab": f16in("sintab", [128, S // 128, HALF]),
        "qnw": f16in("qnw", [128, HD]),
        "knw": f16in("knw", [128, HD]),
        "maskt": f16in("maskt", [128, NHL, 2, CH]),
        "onec": f16in("onec", [128, 1]),
        "qdecb": f16in("qdecb", [128, 2, NHL, 128]),
        "kdec": f32in("kdec", [128, NHL, 2]),
        "lamv": f16in("lamv", [128, NHL]),
        "st0": f16in("st0", [128, NHL, HD]),
        "outp": nc.dram_tensor("outp", [S, HID], F16,
                               kind="ExternalOutput").ap(),
    }
    if DEBUG_TAPS:
        for nm, shape, dtp in [("dbg_q", [128, SUBS, DOUT], F16),
                               ("dbg_k", [128, SUBS, DOUT], F16),
                               ("dbg_v", [128, SUBS, DOUT], F16),
                               ("dbg_og", [128, NHL, BLK], F16),
                               ("dbg_st", [128, NHL, HD], F32),
                               ("dbg_sg", [128, NHL, BLK], F16)]:
            d[nm] = nc.dram_tensor(nm, shape, dtp,
                                   kind="ExternalOutput").ap()

    with tile.TileContext(nc) as tc, ExitStack() as ctx, \
            nc.allow_low_precision(reason="fp16 operands, fp32 accumulate"):
        _body(nc, tc, ctx, d)

    nc.compile()
    return nc


def _body(nc, tc, ctx, d):
    P = 128

    pool = lambda name, bufs: ctx.enter_context(
        tc.tile_pool(name=name, bufs=bufs))
    const = pool("const", 1)      # tables, masks, state (~11k)
    wpool = pool("wpool", 1)      # 80k: resident weights (fp16)
    htpool = pool("ht", 3)        # 48k: hT big tiles, 3 blocks resident
    qkp = pool("qkp", 5)          # 10k: q/k token-major
    vp = pool("vp", 8)            # 8k: v token-major, 2 blocks
    sigp = pool("sigp", 2)        # 8k: sigmoid(g) head-dim-major (fp16)
    sqscp = pool("sqsc", 1)       # 1k: sumsq squares scratch (fp16)
    ropep = pool("ropep", 1)      # 1k: rope m1..m4 (fp16)
    ssp = pool("ssp", 6)          # ~0.5k: sumsq/rstd chains (fp32)
    trp = pool("trp", 2)          # 12k: qT/kT/qdT chunk tiles (fp16)
    stp = pool("stp", 2)          # 2k: masked scoresT (fp16)
    smallp = pool("smallp", 3)    # 3k: kdec-scaled v (fp16)
    stcp = pool("stcp", 2)        # 2k: fp16 state snapshot
    stlp = pool("stlp", 2)        # 2k: decayed state (fp16)
    sqp = pool("sqp", 3)          # 3k: o squares (2 alive per chunk)
    gnp = pool("gnp", 2)          # <1k: group-norm rstd chain
    ogp = pool("ogp", 2)          # 8k: ogT fp16 block
    outsp = pool("outs", 3)       # 6k: dense output staging (fp16)

    psum = ctx.enter_context(tc.tile_pool(name="ps", bufs=8, space="PSUM"))
    psn = [0]

    def ps_tile(shape, dtype=F32):
        psn[0] += 1
        return psum.tile(shape, dtype, tag="ps", name=f"ps{psn[0]}")

    def loadc(name, shape, dtype=F16):
        t = const.tile(shape, dtype, tag=name, name=name)
        nc.scalar.dma_start(t[:], d[name])
        return t

    # HAM warmup: real matmuls on a zeroed scratch tile (no DMA dependency)
    # keep the PE busy during the initial weight DMA so the clock-gate opens
    # (1.2 -> 2.4 GHz) before the first real matmul
    scratch = const.tile([P, 256], F16, tag="scratch", name="scratch")
    nc.vector.memset(scratch[:], 0.0)
    wrm1 = ps_tile([P, 256])
    wrm2 = ps_tile([P, 256])
    for i in range(8):
        nc.tensor.matmul(wrm1[:], scratch[:, 0:128], scratch[:],
                         start=(i == 0), stop=(i == 7))
        nc.tensor.matmul(wrm2[:], scratch[:, 0:128], scratch[:],
                         start=(i == 0), stop=(i == 7))

    # big hT tiles: block 0 filled by per-k DMAs interleaved with the weight
    # stream (fine-grained deps for the prologue); blocks 1,2 one-shot
    ht_big = [htpool.tile([P, KT, BLK], F16, tag="htb", name=f"htb{i}")
              for i in range(3)]
    w_tiles = []
    for k in range(KT):
        wt = wpool.tile([P, 4 * DOUT], F16, tag=f"w{k}", name=f"w{k}")
        nc.sync.dma_start(wt[:], d["wqkvg"][:, k, :])
        w_tiles.append(wt)
        nc.sync.dma_start(ht_big[0][:, k, :],
                          d["hT"][k * 128:(k + 1) * 128, 0:BLK])
    # small consts on the scalar ring - parallel descriptor programming,
    # all landed before the first norm/attention consumers
    cos_t = loadc("costab", [P, S // 128, HALF])
    sin_t = loadc("sintab", [P, S // 128, HALF])
    qnw_t = loadc("qnw", [P, HD])
    knw_t = loadc("knw", [P, HD])
    mask_t = loadc("maskt", [P, NHL, 2, CH])
    qdecb_t = loadc("qdecb", [P, 2, NHL, 128])
    kdec_t = loadc("kdec", [P, NHL, 2], F32)
    lamv_t = loadc("lamv", [P, NHL])
    st_c0 = loadc("st0", [P, NHL, HD])
    ones_col = loadc("onec", [P, 1])
    # bulk prefetch of later hT blocks on the sync ring
    def emit_ht_block(nn, buf):
        nc.sync.dma_start(
            buf[:], d["hT"].rearrange("(k p) s -> p k s", p=128)
            [:, :, nn * BLK:(nn + 1) * BLK])
        return [buf[:, k, :] for k in range(KT)]

    ht_blocks = [None] * NBLK
    ht_blocks[0] = [ht_big[0][:, k, :] for k in range(KT)]
    wd_t = wpool.tile([P, NHL, HID], F16, tag="wd", name="wd")

    def make_proj(ht):
        """Allocate a block's q/k/v tiles; return 12 emit-closures (one PSUM
        accumulation group each: 16 matmuls + eviction + q/k sumsq)."""
        q_blk = [qkp.tile([P, DOUT], F16, tag="qb", name="qb") for _ in range(SUBS)]
        k_blk = [qkp.tile([P, DOUT], F16, tag="kb", name="kb") for _ in range(SUBS)]
        v_blk = [vp.tile([P, DOUT], F16, tag="vb", name="vb") for _ in range(SUBS)]
        ss_l = [ssp.tile([P, 8], F32, tag="ss", name="ss") for _ in range(SUBS)]
        groups = []
        sqg = []
        for sub in range(SUBS):
            for ti, dest in enumerate((q_blk, k_blk, v_blk)):
                def grp(sub=sub, ti=ti, dest=dest, ht=ht):
                    ps = ps_tile([P, DOUT])
                    for k in range(KT):
                        nc.tensor.matmul(
                            ps[:], ht[k][:, sub * 128:(sub + 1) * 128],
                            w_tiles[k][:, ti * DOUT:(ti + 1) * DOUT],
                            start=(k == 0), stop=(k == KT - 1))
                    sb = dest[sub]
                    nc.scalar.copy(sb[:], ps[:])
                groups.append(grp)
                if ti < 2:
                    # sumsq deferred to the end of the filler list: these
                    # vector ops are only needed at the next block's norm,
                    # and emitting them late keeps the vector queue clear
                    # for attention-critical masks/gates
                    def sq(sub=sub, ti=ti, dest=dest, ss_t=ss_l[sub]):
                        sb = dest[sub]
                        sqs = sqscp.tile([P, DOUT], F16, tag="sqscratch")
                        nc.vector.tensor_mul(sqs[:], sb[:], sb[:])
                        nc.vector.tensor_reduce(
                            ss_t[:, ti * 4:ti * 4 + 4],
                            sqs[:].rearrange("p (h d) -> p h d", h=NHL),
                            mybir.AxisListType.X, AL.add)
                    sqg.append(sq)
        return groups + sqg, (q_blk, k_blk, v_blk, ss_l)

    def emit_norm_rope(n, blk_state, subs_range=None):
        q_blk, k_blk, v_blk, ss_l = blk_state
        for sub in (range(SUBS) if subs_range is None else subs_range):
            gs = n * SUBS + sub
            rtmp = ssp.tile([P, 8], F32, tag="rstdt")
            nc.vector.tensor_scalar(rtmp[:], ss_l[sub][:], 1.0 / HD, EPS,
                                    AL.mult, AL.add)
            nc.vector.reciprocal(rtmp[:], rtmp[:])
            rstd_t = ssp.tile([P, 8], F16, tag="rstd")
            nc.scalar.activation(rstd_t[:], rtmp[:], AF.Sqrt)
            for ti, (blk, nw_t) in enumerate(((q_blk, qnw_t), (k_blk, knw_t))):
                x = blk[sub]
                x3 = x.rearrange("p (h d) -> p h d", h=NHL)
                rsl = rstd_t[:, ti * 4:ti * 4 + 4]
                nc.vector.tensor_mul(
                    x3, x3, rsl.unsqueeze(2).to_broadcast((P, NHL, HD)))
                nc.vector.tensor_mul(
                    x3, x3, nw_t[:].unsqueeze(1).to_broadcast((P, NHL, HD)))
                x1, x2 = x3[:, :, 0:HALF], x3[:, :, HALF:ROT]
                cosb = cos_t[:, gs, :].unsqueeze(1).to_broadcast(
                    (P, NHL, HALF))
                sinb = sin_t[:, gs, :].unsqueeze(1).to_broadcast(
                    (P, NHL, HALF))
                m1 = ropep.tile([P, NHL, HALF], F16, tag="m1")
                m2 = ropep.tile([P, NHL, HALF], F16, tag="m2")
                m3 = ropep.tile([P, NHL, HALF], F16, tag="m3")
                m4 = ropep.tile([P, NHL, HALF], F16, tag="m4")
                nc.vector.tensor_mul(m1[:], x1, cosb)
                nc.vector.tensor_mul(m2[:], x2, sinb)
                nc.vector.tensor_mul(m3[:], x2, cosb)
                nc.vector.tensor_mul(m4[:], x1, sinb)
                nc.vector.tensor_sub(x1, m1[:], m2[:])
                nc.vector.tensor_add(x2, m3[:], m4[:])

    def alloc_chunk_T():
        # [d, si, h, t] head-dim-major chunk tiles, filled by XBAR transposes
        qT = trp.tile([P, 2, NHL, 128], F16, tag="qT", name="qT")
        kT = trp.tile([P, 2, NHL, 128], F16, tag="kT", name="kT")
        return qT, kT

    def emit_xpose(blk_state, cc_tiles, sub):
        # XBAR DMA transpose (scalar ring): q_blk[sub] [t,(h d)] -> [d,h,t]
        q_blk, k_blk, v_blk, ss_l = blk_state
        qT, kT = cc_tiles[sub // 2]
        si = sub % 2
        nc.sync.dma_start_transpose(qT[:, si], q_blk[sub][:])
        nc.scalar.dma_start_transpose(kT[:, si], k_blk[sub][:])

    def dense_group(nn, ogT, rstd_gn, sub, dm):
        # contraction split per head-group so the per-(token, group) rstd
        # lands as a per-partition scalar on the eviction; psA eviction on
        # scalar, psB fuse on vector, one output DMA per group
        psA = ps_tile([P, BLK])
        psB = ps_tile([P, BLK])
        for kk in (0, 1):
            nc.tensor.matmul(
                psA[:], ogT[:, kk, sub * 128:(sub + 1) * 128],
                wd_t[:, kk, dm * BLK:(dm + 1) * BLK],
                start=(kk == 0), stop=(kk == 1))
        for kk in (2, 3):
            nc.tensor.matmul(
                psB[:], ogT[:, kk, sub * 128:(sub + 1) * 128],
                wd_t[:, kk, dm * BLK:(dm + 1) * BLK],
                start=(kk == 2), stop=(kk == 3))
        rows = slice(nn * BLK + sub * 128, nn * BLK + (sub + 1) * 128)
        tmp = outsp.tile([P, BLK], F16, tag="ostmp")
        nc.scalar.activation(tmp[:], psA[:], AF.Copy,
                             scale=rstd_gn[:, sub, 0:1])
        osb = outsp.tile([P, BLK], F16, tag="osb")
        nc.vector.scalar_tensor_tensor(osb[:], psB[:],
                                       rstd_gn[:, sub, 1:2], tmp[:],
                                       AL.mult, AL.add)
        nc.sync.dma_start(d["outp"][rows, dm * BLK:(dm + 1) * BLK], osb[:])

    # prologue: block 0 projections emitted directly, with each sub's
    # norm+rope and XBAR transposes emitted right after its three groups
    # block-0 q/k projections k-outer across 8 PSUM banks: each weight
    # tile is consumed as it lands, so the PE keeps pace with the 8MB
    # weight DMA instead of stalling on per-group k-inner accumulation
    q_blk0 = [qkp.tile([P, DOUT], F16, tag="qb", name="qb") for _ in range(SUBS)]
    k_blk0 = [qkp.tile([P, DOUT], F16, tag="kb", name="kb") for _ in range(SUBS)]
    v_blk0 = [vp.tile([P, DOUT], F16, tag="vb", name="vb") for _ in range(SUBS)]
    ss_l0 = [ssp.tile([P, 8], F32, tag="ss", name="ss") for _ in range(SUBS)]
    cur = (q_blk0, k_blk0, v_blk0, ss_l0)
    ht0 = ht_blocks[0]
    ps_qk = [[ps_tile([P, DOUT]) for _ti in range(2)] for _s in range(SUBS)]
    for k in range(KT):
        for sub in range(SUBS):
            for ti in range(2):
                nc.tensor.matmul(
                    ps_qk[sub][ti][:], ht0[k][:, sub * 128:(sub + 1) * 128],
                    w_tiles[k][:, ti * DOUT:(ti + 1) * DOUT],
                    start=(k == 0), stop=(k == KT - 1))
    ccT0 = [alloc_chunk_T(), alloc_chunk_T()]
    for sub in range(SUBS):
        for ti, dest in ((0, q_blk0), (1, k_blk0)):
            sb = dest[sub]
            nc.scalar.copy(sb[:], ps_qk[sub][ti][:])
            sqs = sqscp.tile([P, DOUT], F16, tag="sqscratch")
            nc.vector.tensor_mul(sqs[:], sb[:], sb[:])
            nc.vector.tensor_reduce(
                ss_l0[sub][:, ti * 4:ti * 4 + 4],
                sqs[:].rearrange("p (h d) -> p h d", h=NHL),
                mybir.AxisListType.X, AL.add)
        emit_norm_rope(0, cur, subs_range=(sub,))
        emit_xpose(cur, ccT0, sub)
        # v projection (k-inner, weights resident by now) keeps the PE
        # busy while the vector engine norms/ropes
        ps_v = ps_tile([P, DOUT])
        for k in range(KT):
            nc.tensor.matmul(ps_v[:], ht0[k][:, sub * 128:(sub + 1) * 128],
                             w_tiles[k][:, 2 * DOUT:3 * DOUT],
                             start=(k == 0), stop=(k == KT - 1))
        nc.scalar.copy(v_blk0[sub][:], ps_v[:])
        if sub == 3:
            ht_blocks[1] = emit_ht_block(1, ht_big[1])
    ccT_cur = ccT0

    filler = []
    reserve = []  # dense groups held for the last block's attention phase
    st_cur = [st_c0]  # fp16 recurrent state, replaced each chunk

    def drain(k):
        for _ in range(min(k, len(filler))):
            filler.pop(0)()

    for n in range(NBLK):
        q_blk, k_blk, v_blk, ss_l = cur
        drain(len(filler))  # leftover dense from previous block

        ccT_nxt = None
        if n > 0:
            emit_norm_rope(n, cur)
            for sub in range(SUBS):
                emit_xpose(cur, ccT_cur, sub)

        # g projection emitted lazily (interleaved into attention)
        sig_blk = sigp.tile([P, NHL, BLK], F16, tag="sig")

        def g_proj_group(mg, ht=ht_blocks[n], sig=sig_blk):
            ps = ps_tile([P, BLK])
            for k in range(KT):
                nc.tensor.matmul(
                    ps[:],
                    w_tiles[k][:, 3 * DOUT + mg * 128:3 * DOUT + (mg + 1) * 128],
                    ht[k], start=(k == 0), stop=(k == KT - 1))
            nc.scalar.activation(sig[:, mg, :], ps[:], AF.Sigmoid)

        filler.extend([lambda mg=mg: g_proj_group(mg) for mg in range(NHL)])

        # enqueue next block's projection groups as filler
        if n + 1 < NBLK:
            groups, nxt = make_proj(ht_blocks[n + 1])
            filler.extend(groups)
        else:
            nxt = None
            filler.extend(reserve)
            reserve.clear()
        if n == 0:
            nc.sync.dma_start(wd_t[:], d["wd"])
            ht_blocks[2] = emit_ht_block(2, ht_big[2])
        drain(2)

        if DEBUG_TAPS and n == 0:
            for sub in range(SUBS):
                if "q" in TAP_SET:
                    nc.sync.dma_start(d["dbg_q"][:, sub, :], q_blk[sub][:])
                if "k" in TAP_SET:
                    nc.sync.dma_start(d["dbg_k"][:, sub, :], k_blk[sub][:])
                if "v" in TAP_SET:
                    nc.sync.dma_start(d["dbg_v"][:, sub, :], v_blk[sub][:])

        # ---- attention: 2 chunks of 256, head-pair batched ----
        ogT_blk = ogp.tile([P, NHL, BLK], F16, tag="ogT")
        rstd_gn = gnp.tile([P, SUBS, 2], F32, tag="grstd")
        for cc in range(2):
            subs = (2 * cc, 2 * cc + 1)
            qT_all, kT_all = ccT_cur[cc]

            st_c = st_cur[0]
            st_new = stcp.tile([P, NHL, HD], F16, tag="stc")
            st_lam = stlp.tile([P, NHL, HD], F16, tag="stlam")
            nc.vector.tensor_mul(
                st_lam[:], st_c[:],
                lamv_t[:].unsqueeze(2).to_broadcast((P, NHL, HD)))
            vd_c = []
            for si, sub in enumerate(subs):
                vd = smallp.tile([P, NHL, HD], F16, tag="vd")
                nc.vector.tensor_mul(
                    vd[:], v_blk[sub][:].rearrange("p (h e) -> p h e", h=NHL),
                    kdec_t[:, :, si].unsqueeze(2).to_broadcast((P, NHL, HD)))
                vd_c.append(vd)
            qdT_all = trp.tile([P, 2, NHL, 128], F16, tag="qdT")
            nc.vector.tensor_mul(qdT_all[:], qT_all[:], qdecb_t[:])
            # block 0 pays the pipeline-fill bubble of the first attention
            # (evict -> norm/rope -> XBAR transpose chain): feed the PE more
            # filler before the first scores matmul there
            drain(4 if n == 0 else 2)

            # per head-pair (= norm group): scores, o (intra+inter), square,
            # gate - shared PSUM banks, batched evictions
            sq_g = []
            for g in range(2):
                sT = []
                for hh in range(2):
                    h = 2 * g + hh
                    pst = ps_tile([P, 2 * CH])
                    for si in range(2):
                        nc.tensor.matmul(pst[:, si * CH:(si + 1) * CH],
                                         kT_all[:, si, h, :],
                                         qT_all[:, :, h, :],
                                         start=True, stop=True)
                    st = stp.tile([P, 2, CH], F16, tag="sT")
                    nc.vector.tensor_mul(
                        st[:], pst[:].rearrange("p (s c) -> p s c", s=2),
                        mask_t[:, h, :, :])
                    sT.append(st)
                drain(1)
                o_ps = ps_tile([P, 2, CH])
                for hh in range(2):
                    h = 2 * g + hh
                    for si, sub in enumerate(subs):
                        nc.tensor.matmul(
                            o_ps[:, hh, :],
                            v_blk[sub][:, h * HD:(h + 1) * HD],
                            sT[hh][:, si, :], start=(si == 0), stop=False)
                    nc.tensor.matmul(o_ps[:, hh, :], st_c[:, h, :],
                                     qdT_all[:, :, h, :],
                                     start=False, stop=True)
                # scale 1/64 before squaring: o can reach ~1e3 for
                # weak-decay heads and o^2 would overflow fp16
                sq = sqp.tile([P, 2, CH], F16, tag="sq")
                nc.scalar.activation(sq[:], o_ps[:], AF.Square,
                                     scale=1.0 / 64.0)
                nc.vector.tensor_mul(
                    ogT_blk[:, 2 * g:2 * g + 2, cc * CH:(cc + 1) * CH],
                    o_ps[:], sig_blk[:, 2 * g:2 * g + 2, cc * CH:(cc + 1) * CH])
                sq_g.append(sq)
                drain(1)

            # state update: all 4 heads share one PSUM bank; decay term
            # applied on the vector engine, fused into the eviction add
            dl_ps = ps_tile([P, NHL, HD])
            for h in range(NHL):
                for si, sub in enumerate(subs):
                    nc.tensor.matmul(
                        dl_ps[:, h, :], k_blk[sub][:, h * HD:(h + 1) * HD],
                        vd_c[si][:, h, :], start=(si == 0), stop=(si == 1))
                if h == 1:
                    drain(1)
            nc.vector.tensor_add(st_new[:], dl_ps[:], st_lam[:])
            st_cur[0] = st_new
            drain(1)

            # group norm rstd, batched [P, 4] = (si, g): head-pair sums
            # pre-added on vector, then sq (hd-major) x ones -> [tokens, 1]
            gsum = []
            for g in range(2):
                gs = sqp.tile([P, CH], F16, tag="gsum")
                nc.vector.tensor_add(gs[:], sq_g[g][:, 0, :],
                                     sq_g[g][:, 1, :])
                gsum.append(gs)
            gcol = ps_tile([P, 4])
            for si in range(2):
                for g in range(2):
                    nc.tensor.matmul(
                        gcol[:, si * 2 + g:si * 2 + g + 1],
                        gsum[g][:, si * 128:(si + 1) * 128],
                        ones_col[:], start=True, stop=True)
            grt = gnp.tile([P, 4], F32, tag="grt")
            nc.vector.tensor_scalar(grt[:], gcol[:], 4096.0 / (2 * HD), EPS,
                                    AL.mult, AL.add)
            nc.vector.reciprocal(grt[:], grt[:])
            nc.scalar.activation(
                rstd_gn[:, 2 * cc:2 * cc + 2, :].rearrange(
                    "p a b -> p (a b)"), grt[:], AF.Sqrt)
            drain(1)

            # interleave dense groups 1:1 into the remaining filler (proj
            # groups): a contiguous dense burst stalls on evictions
            dns = [lambda nn=n, og=ogT_blk, rs=rstd_gn, s=sub, m=dm:
                   dense_group(nn, og, rs, s, m)
                   for sub in (2 * cc, 2 * cc + 1)
                   for dm in range(HID // BLK)]
            if n == NBLK - 2:
                # hold back dense work to cover the last block's attention
                reserve.extend(dns[2:])
                dns = dns[:2]
            mixed = []
            while filler or dns:
                if filler:
                    mixed.append(filler.pop(0))
                if dns:
                    mixed.append(dns.pop(0))
            filler[:] = mixed
            drain(2)
            if n == 1 and cc == 0:
                # late prefetch of the last hT block into block-0's buffer
                # (sync ring; WAR on block-0 g-proj resolves early in this
                # block's attention)
                ht_blocks[3] = emit_ht_block(3, ht_big[0])

        if DEBUG_TAPS and n == 0:
            if "sg" in TAP_SET:
                nc.sync.dma_start(d["dbg_sg"], sig_blk[:])
            if "og" in TAP_SET:
                nc.sync.dma_start(d["dbg_og"], ogT_blk[:])
            if "st" in TAP_SET:
                st_dump = const.tile([P, NHL, HD], F32, tag="stdump")
                nc.vector.tensor_copy(st_dump[:], st_cur[0][:])
                nc.sync.dma_start(d["dbg_st"], st_dump[:])

        if nxt is not None:
            ccT_nxt = [alloc_chunk_T(), alloc_chunk_T()]
        cur = nxt
        ccT_cur = ccT_nxt

    drain(len(filler))


_NC_CACHE = None


def _get_module():
    global _NC_CACHE
    if _NC_CACHE is None:
        _NC_CACHE = _build_module()
    return _NC_CACHE


def _host_inputs(positions, hidden_states, recurrent_state, w_qkv, w_g,
                 w_dense, q_norm_w, k_norm_w, g_norm_w):
    """Build the 8 per-core input dicts."""
    F16NP = np.float16
    positions = np.asarray(positions)
    hidden_states = np.asarray(hidden_states, dtype=np.float32)
    recurrent_state = np.asarray(recurrent_state, dtype=np.float32)
    w_qkv = np.asarray(w_qkv, dtype=np.float32)
    w_g = np.asarray(w_g, dtype=np.float32)
    w_dense = np.asarray(w_dense, dtype=np.float32)
    q_norm_w = np.asarray(q_norm_w, dtype=np.float32)
    k_norm_w = np.asarray(k_norm_w, dtype=np.float32)
    g_norm_w = np.asarray(g_norm_w, dtype=np.float32)

    # rope tables from positions: [S, HALF] -> [128, S//128, HALF]
    inv_freq = 1.0 / (THETA ** (np.arange(HALF, dtype=np.float64) / HALF))
    ang = positions.astype(np.float64)[:, None] * inv_freq[None, :]
    cos = np.cos(ang).reshape(S // 128, 128, HALF).transpose(1, 0, 2)
    sin = np.sin(ang).reshape(S // 128, 128, HALF).transpose(1, 0, 2)
    cos = np.ascontiguousarray(cos.astype(F16NP))
    sin = np.ascontiguousarray(sin.astype(F16NP))

    qnw = np.ascontiguousarray(np.tile(q_norm_w[None, :], (128, 1))
                               .astype(F16NP))
    knw = np.ascontiguousarray(np.tile(k_norm_w[None, :], (128, 1))
                               .astype(F16NP))

    i_idx = np.arange(CH, dtype=np.float64)
    j_idx = np.arange(CH, dtype=np.float64)

    in_maps = []
    for core in range(NCORES):
        b = core // 4
        h0 = (core % 4) * NHL  # global head offset
        cs, ce = h0 * HD, (h0 + NHL) * HD

        hT = np.ascontiguousarray(hidden_states[b].T.astype(F16NP))

        w_all = np.concatenate(
            [w_qkv[:, cs:ce], w_qkv[:, NH * HD + cs:NH * HD + ce],
             w_qkv[:, 2 * NH * HD + cs:2 * NH * HD + ce],
             w_g[:, cs:ce]], axis=1)  # [HID, 4*DOUT]
        wqkvg = np.ascontiguousarray(
            w_all.reshape(KT, 128, 4 * DOUT).transpose(1, 0, 2).astype(F16NP))

        wd = (w_dense[cs:ce, :] * g_norm_w[cs:ce, None]).astype(F16NP)
        wd = np.ascontiguousarray(wd.reshape(NHL, 128, HID).transpose(1, 0, 2))

        loglam = _SLOPE_ALL[h0:h0 + NHL]  # [NHL] negative
        # maskt[p, h, jsub, i] = lam^(i - (jsub*128+p)) for i >= j else 0
        mm = np.where(i_idx[None, None, :] >= j_idx[None, :, None],
                      np.exp(loglam[:, None, None]
                             * (i_idx[None, None, :] - j_idx[None, :, None])),
                      0.0)  # [NHL, j, i]
        maskt = np.ascontiguousarray(
            mm.reshape(NHL, 2, 128, CH).transpose(2, 0, 1, 3).astype(F16NP))

        # qdec as a partition-broadcast table [p, si, h, t] = lam_h^(i+1),
        # i = si*128 + t; applied to qT ([d, si, h, t]) after the transpose
        qdec = np.exp(loglam[None, :, None] * (
            (i_idx.reshape(2, 1, 128)) + 1.0))  # [2, NHL, 128]
        qdecb = np.ascontiguousarray(
            np.broadcast_to(qdec[None].astype(F16NP), (128, 2, NHL, 128)))
        kdec = np.exp(loglam[:, None] * (CH - 1.0 - j_idx[None, :]))
        kdec = np.ascontiguousarray(
            kdec.reshape(NHL, 2, 128).transpose(2, 0, 1).astype(np.float32))
        lamv = np.ascontiguousarray(np.broadcast_to(
            np.exp(loglam * CH).astype(F16NP)[None, :], (128, NHL)))

        st0 = np.ascontiguousarray(
            recurrent_state[b, h0:h0 + NHL].transpose(1, 0, 2))  # [d, h, e]

        in_maps.append({
            "onec": np.ones((128, 1), F16NP),
            "hT": hT, "wqkvg": wqkvg, "wd": wd,
            "costab": cos, "sintab": sin, "qnw": qnw, "knw": knw,
            "maskt": maskt, "qdecb": qdecb, "kdec": kdec,
            "lamv": lamv,
            "st0": st0.astype(F16NP),
        })
    return in_maps


def kernel(**inputs):
    nc = _get_module()
    in_maps = _host_inputs(**inputs)
    res = run_bass_kernel_spmd(nc, in_maps, core_ids=list(range(NCORES)))
    outs = [r["outp"] for r in res.results]
    out = np.stack([outs[0] + outs[1] + outs[2] + outs[3],
                    outs[4] + outs[5] + outs[6] + outs[7]])
    return out.astype(np.float32)
